# revision 31
# baseline (speedup 1.0000x reference)
"""MDTA (Restormer transposed attention) Trainium2 Bass kernel.

Strategy: data-parallel over batch (8 cores x 1 image each). Per core:
  q,k path runs in fp8e4m3 with DoubleRow perf mode (0.5 cycles/row):
    1. qkv GEMM for q,k: one DR matmul per 512-col tile (K=192 packed as
       [128, 2, .] subtiles), weights x4 so fp8 stays in normal range.
    2. depthwise 3x3 for q,k: 9 taps paired into 5 DR diag matmuls per tile
       (pair deltas must be even), reading a zero-margin padded pre buffer;
       row-wrap contamination of dx=+-1 taps fixed by strided DVE STTs.
    3. streaming per 2-block pair: fp8 transposes (element-step-2 psum out)
       -> packed [128,2,192] q/k pair tiles -> DR matmuls accumulate raw
       attention logits AND q/k Gram matrices (norms = Gram diag, so all
       fp8/weight scalings cancel in q_hat/k_hat automatically).
    4. logits = G * rsq[c] * rsk[d] (row TSP + PE-broadcast column scale),
       mask + softmax, M^T = A^T w_outT (f32 matmuls).
  v path stays bf16 (its error reaches the output directly):
    5. v GEMM from resident bf16 x (K=64 tail streamed from DRAM),
       depthwise via diag matmuls on PE with some tiles offloaded to
       DVE/Pool STT chains, then y = M^T.T @ v_dw as one GEMM over v.
"""
import sys
sys.path.insert(0, "/opt/trn_rl_repo")
from contextlib import ExitStack

import numpy as np
import ml_dtypes

import concourse.bass as bass
import concourse.mybir as mybir
import concourse.tile as tile
from concourse import bacc
from concourse.bass_utils import run_bass_kernel_spmd

F32 = mybir.dt.float32
BF16 = mybir.dt.bfloat16
FP8 = mybir.dt.float8e4
AF = mybir.ActivationFunctionType
ALU = mybir.AluOpType
AX = mybir.AxisListType
DR = mybir.MatmulPerfMode.DoubleRow

C = 192
NHEADS = 8
HDIM = 24
H = W = 128
N = H * W            # 16384 spatial positions
NT = 512             # free-dim tile (4 image rows)
NTILES = N // NT     # 32
PAD = 256            # zero margin on each side of padded pre buffers
NPAD = N + 2 * PAD
EPS = 1e-12

# taps: index t = (dy+1)*3 + (dx+1), shift = dy*W + dx
TAPS = [(dy, dx) for dy in (-1, 0, 1) for dx in (-1, 0, 1)]
SHIFT = [dy * W + dx for dy, dx in TAPS]
# fp8 DR tap pairs (subtile deltas must be EVEN for walrus/HW):
#   (t_a, t_b) -> rhs offset SHIFT[t_a], delta SHIFT[t_b]-SHIFT[t_a]
QK_PAIRS = [(0, 2), (3, 5), (6, 8), (1, 4), (7, None)]

# engine plan (tunable): which engine handles each copy/offload site.
# NOTE: Pool/GPSIMD cannot access PSUM on HW -> psum->sbuf copies are
# ACT/DVE only; Pool gets SBUF-only work (STT dw chains, memsets).
PLAN = {
    "qk_pre": ["act", "dve"],            # striped by tile index
    "qk_dw": ["dve", "act"],
    "pair_q0": "act",                    # chunk0 pair pack copy [128,2,128]
    "pair_c1a": "act",                   # chunk1 q split copy [128,2,64]
    "pair_c1b": "dve",                   # chunk1 k split copy [128,2,64]
    "pair_k2": "dve",                    # chunk2 pair pack copy
    "v_pre": ["act", "act"],             # ACT (DVE runs dw trees)
    "v_dw_copy": ["act", "act"],
    "y_copy": ["act", "act"],
    # v dw tile offload: Pool lacks TensorScalarPtr on HW -> PE/DVE only
    "v_dw": lambda ci, t: ("dve" if t % 3 == 1 else "pe"),
    "pipe_depth": 2,                     # attn pair software-pipeline depth
    "dup4": False,                       # chunk4 tap-pairing via shifted dup
}


def build_nc(reps=1, abl=(), plan=PLAN):  # noqa: C901
    nc = bacc.Bacc("TRN2", target_bir_lowering=False, debug=False)
    x_d = (nc.dram_tensor("x_scratch", [C, N], F32) if "dummyx" in abl
           else nc.dram_tensor("x", [C, N], F32, kind="ExternalInput"))
    xt_d = (nc.dram_tensor("xt_scratch", [64, N], BF16) if "dummyx" in abl
            else nc.dram_tensor("xt16", [64, N], BF16, kind="ExternalInput"))
    wq8_d = nc.dram_tensor("wq8", [128, 768], FP8, kind="ExternalInput")
    wqv0_d = nc.dram_tensor("wqv0", [128, 192], BF16, kind="ExternalInput")
    wqv1_d = nc.dram_tensor("wqv1", [64, 192], BF16, kind="ExternalInput")
    wdw8_d = nc.dram_tensor("wdw8", [3, 128, 1280], FP8, kind="ExternalInput")
    wdwv_d = nc.dram_tensor("wdwv", [2, 128, 1152], BF16, kind="ExternalInput")
    wdv4p_d = nc.dram_tensor("wdv4p", [128, 384], BF16, kind="ExternalInput")
    wdn_d = nc.dram_tensor("wdn", [128, 45], F32, kind="ExternalInput")
    wdvp_d = nc.dram_tensor("wdvp", [128, 18], F32, kind="ExternalInput")
    wo_d = nc.dram_tensor("w_outT", [C, C], F32, kind="ExternalInput")
    tmp_d = nc.dram_tensor("temp", [C, 1], F32, kind="ExternalInput")
    mask_d = nc.dram_tensor("mask", [C, C], F32, kind="ExternalInput")
    eye8_d = nc.dram_tensor("eye8", [128, 128], FP8, kind="ExternalInput")
    eyeb_d = nc.dram_tensor("eyeb", [128, 128], BF16, kind="ExternalInput")
    if "dummyy" in abl:
        y_d = nc.dram_tensor("y_scratch", [C, N], F32)
        yprobe_d = nc.dram_tensor("y", [128, 4], F32, kind="ExternalOutput")
    else:
        y_d = nc.dram_tensor("y", [C, N], F32, kind="ExternalOutput")
        yprobe_d = None

    def cp(eng, out, in_):
        if eng == "act":
            nc.scalar.copy(out, in_)
        elif eng == "dve":
            nc.vector.tensor_copy(out, in_)
        else:
            nc.gpsimd.tensor_copy(out, in_)

    with tile.TileContext(nc) as tc, ExitStack() as ctx:
        wp = ctx.enter_context(tc.tile_pool(name="w", bufs=1))
        sp = ctx.enter_context(tc.tile_pool(name="small", bufs=1))
        yp = ctx.enter_context(tc.tile_pool(name="y", bufs=3))

        # ---- persistent weights ----
        wq8 = wp.tile([128, 2, 384], FP8, tag="wq8")
        nc.sync.dma_start(wq8[:].rearrange("p a b -> p (a b)"), wq8_d[:])
        wqv0 = wp.tile([128, 192], BF16, tag="wqv0")
        wqv1 = wp.tile([64, 192], BF16, tag="wqv1")
        nc.sync.dma_start(wqv0[:], wqv0_d[:])
        nc.sync.dma_start(wqv1[:], wqv1_d[:])
        wdw8 = []
        for ci in range(3):
            t_ = wp.tile([128, 5, 2, 128], FP8, tag=f"wdw8_{ci}", name=f"wdw8_{ci}")
            nc.sync.dma_start(t_[:].rearrange("p a b c -> p (a b c)"), wdw8_d[ci])
            wdw8.append(t_)
        wdv3 = wp.tile([128, 1152], BF16, tag="wdv3")
        nc.sync.dma_start(wdv3[:], wdwv_d[0])
        wdv4 = wp.tile([64, 1152], BF16, tag="wdv4")
        nc.sync.dma_start(wdv4[:], wdwv_d[1, 0:64, :])
        wdv4p = wp.tile([128, 384], BF16, tag="wdv4p")
        nc.sync.dma_start(wdv4p[:], wdv4p_d[:])
        wdn = wp.tile([128, 45], F32, tag="wdn")
        nc.sync.dma_start(wdn[:], wdn_d[:])
        wdvp = wp.tile([128, 18], F32, tag="wdvp")
        nc.sync.dma_start(wdvp[:], wdvp_d[:])
        wo0 = wp.tile([128, C], F32, tag="wo0")
        wo1 = wp.tile([64, C], F32, tag="wo1")
        nc.sync.dma_start(wo0[:], wo_d[0:128, :])
        nc.sync.dma_start(wo1[:], wo_d[128:192, :])
        tmp0 = wp.tile([128, 1], F32, tag="tmp0")
        tmp1 = wp.tile([64, 1], F32, tag="tmp1")
        nc.sync.dma_start(tmp0[:], tmp_d[0:128, :])
        nc.sync.dma_start(tmp1[:], tmp_d[128:192, :])
        mask0 = wp.tile([128, C], F32, tag="mask0")
        mask1 = wp.tile([64, C], F32, tag="mask1")
        nc.sync.dma_start(mask0[:], mask_d[0:128, :])
        nc.sync.dma_start(mask1[:], mask_d[128:192, :])
        eye8 = wp.tile([128, 128], FP8, tag="eye8")
        nc.sync.dma_start(eye8[:], eye8_d[:])
        eyeb = wp.tile([128, 128], BF16, tag="eyeb")
        nc.sync.dma_start(eyeb[:], eyeb_d[:])
        onesb = wp.tile([1, 128], BF16, tag="onesb")
        nc.vector.memset(onesb[:], 1.0)
        # M^T tiles (built in phase 1, used in phase 2)
        mt0 = wp.tile([128, C], BF16, tag="mt0")
        mt1 = wp.tile([64, C], BF16, tag="mt1")

        # resident x: bf16 (v path) + fp8 DR-packed (q,k path), loaded once
        xres0 = wp.tile([128, N], BF16, tag="xres0")
        xres8 = wp.tile([128, 2, N], FP8, tag="xres8")
        nc.gpsimd.memset(xres8[64:128, 1, :], 0.0)
        with tc.tile_pool(name="xload", bufs=3) as xp:
            for t in range(NTILES):
                cols = slice(t * NT, (t + 1) * NT)
                xt0 = xp.tile([128, NT], F32, tag="x0")
                xt1 = xp.tile([64, NT], F32, tag="x1")
                nc.sync.dma_start(xt0[:], x_d[0:128, cols])
                nc.sync.dma_start(xt1[:], x_d[128:192, cols])
                nc.vector.tensor_copy(xres0[:, cols], xt0[:])
                nc.scalar.copy(xres8[:, 0, cols], xt0[:])
                nc.gpsimd.tensor_copy(xres8[0:64, 1, cols], xt1[:])

        for _rep in range(reps):
         # ================= phase 1: q,k in fp8 =================
         with tc.tile_pool(name="pre8p", bufs=1) as pre8p, \
             tc.tile_pool(name="dw8p", bufs=1) as dw8p, \
             tc.tile_pool(name="qkp", bufs=4) as qkp, \
             tc.tile_pool(name="asb", bufs=1) as ap_:

            pre8 = pre8p.tile([128, NPAD], FP8, tag="pre8")
            nc.gpsimd.memset(pre8[:, 0:PAD], 0.0)
            nc.gpsimd.memset(pre8[:, PAD + N:NPAD], 0.0)
            dw8 = [dw8p.tile([128, N], FP8, tag=f"dw8_{ci}", name=f"dw8_{ci}")
                   for ci in range(3)]

            qk_psum = ExitStack()
            with qk_psum:
             gps = qk_psum.enter_context(
                 tc.tile_pool(name="gps1", bufs=2, space="PSUM"))
             dps = qk_psum.enter_context(
                 tc.tile_pool(name="dps1", bufs=2, space="PSUM"))
             for ci in range(3):
                # GEMM: one DR matmul per tile (K=192 packed)
                for t in range(NTILES):
                    cols = slice(t * NT, (t + 1) * NT)
                    pg = gps.tile([128, NT], F32, tag="g")
                    nc.tensor.matmul(pg[:], wq8[:, :, ci * 128:(ci + 1) * 128],
                                     xres8[:, :, cols], start=True, stop=True,
                                     perf_mode=DR)
                    cp(plan["qk_pre"][t % 2],
                       pre8[:, PAD + t * NT:PAD + (t + 1) * NT], pg[:])
                # depthwise: 5 DR matmuls per tile (paired taps)
                for t in range(NTILES):
                    base = t * NT
                    pd = dps.tile([128, NT], F32, tag="d")
                    for j, (ta, tb) in enumerate(QK_PAIRS):
                        off = PAD + base + SHIFT[ta]
                        delta = (SHIFT[tb] - SHIFT[ta]) if tb is not None else 2
                        rhs = bass.AP(pre8.tensor, off,
                                      [[NPAD, 128], [delta, 2], [1, NT]])
                        nc.tensor.matmul(pd[:], wdw8[ci][:, j], rhs,
                                         start=(j == 0), stop=(j == 4),
                                         perf_mode=DR)
                    cp(plan["qk_dw"][t % 2], dw8[ci][:, base:base + NT], pd[:])
                # edge fixes: subtract row-wrap contamination of dx=+-1 taps
                for dy in (-1, 0, 1):
                    ti_l = (dy + 1) * 3
                    y0, y1 = max(0, 1 - dy), min(127, 128 - dy)
                    out_ap = dw8[ci][:, y0 * W:y1 * W + 1:W]
                    src_ap = bass.AP(pre8.tensor, PAD + (y0 + dy) * W - 1,
                                     [[NPAD, 128], [W, y1 - y0 + 1]])
                    nc.vector.scalar_tensor_tensor(
                        out=out_ap, in0=src_ap,
                        scalar=wdn[:, ci * 9 + ti_l:ci * 9 + ti_l + 1],
                        in1=out_ap, op0=ALU.mult, op1=ALU.add)
                    ti_r = (dy + 1) * 3 + 2
                    y0, y1 = max(0, -1 - dy), min(127, 126 - dy)
                    out_ap = dw8[ci][:, y0 * W + W - 1:y1 * W + W:W]
                    src_ap = bass.AP(pre8.tensor, PAD + (y0 + dy + 1) * W,
                                     [[NPAD, 128], [W, y1 - y0 + 1]])
                    nc.vector.scalar_tensor_tensor(
                        out=out_ap, in0=src_ap,
                        scalar=wdn[:, ci * 9 + ti_r:ci * 9 + ti_r + 1],
                        in1=out_ap, op0=ALU.mult, op1=ALU.add)

            # ---- streaming transposes + raw attn + Gram accumulation ----
            # 6 concurrent psum accumulators; each needs its OWN bank
            # (interleaved start/stop chains sharing a bank corrupt on HW)
            aps_ctx = ExitStack()
            aps = aps_ctx.enter_context(
                tc.tile_pool(name="aps", bufs=1, space="PSUM"))
            apq0 = aps.tile([128, C], F32, tag="apq0")
            apq1t = aps.tile([64, C], F32, tag="apq1")
            apq1 = apq1t[:]
            gq0 = aps.tile([128, 128], F32, tag="gq0")
            gk0 = aps.tile([128, 128], F32, tag="gk0")

            def pair_matmuls(qk, pb):
                st, sp_ = (pb == 0), (pb == 63)
                q3 = qk[:]
                nc.tensor.matmul(apq0[:], q3[:, :, 0:128], q3[:, :, 192:384],
                                 start=st, stop=sp_, perf_mode=DR,
                                 skip_group_check=True)
                nc.tensor.matmul(apq1, q3[:, :, 128:192], q3[:, :, 192:384],
                                 start=st, stop=sp_, perf_mode=DR,
                                 skip_group_check=True)
                for out, lo, hi in ((gq0, 0, 128), (gk0, 192, 320)):
                    nc.tensor.matmul(out[:], q3[:, :, lo:hi], q3[:, :, lo:hi],
                                     start=st, stop=sp_, perf_mode=DR,
                                     skip_group_check=True)

            junk = ap_.tile([128, 2048], BF16, tag="junk")
            stat = sp.tile([128, 8], F32, tag="stat")
            for s4 in range(8):
                nc.scalar.activation(junk[:], dw8[1][:, s4 * 2048:(s4 + 1) * 2048],
                                     AF.Square, accum_out=stat[:, s4:s4 + 1])
            d1 = sp.tile([128, 1], F32, tag="d1")
            nc.vector.tensor_reduce(d1[:], stat[:], axis=AX.X, op=ALU.add)
            pending = []
            with tc.tile_pool(name="tps", bufs=4, space="PSUM") as tps:
                for pb in range(64):
                    b0 = 2 * pb
                    # one psum tile holds all 3 chunk transposes (1536B<bank)
                    # h-major layout: (h, ci) at offset h*768+ci*256 makes
                    # the pack copy affine over all 384 channels
                    pt = tps.tile([128, 1536], FP8, tag="pt")
                    for ci in range(3):
                        for h in range(2):
                            out_ap = bass.AP(pt.tensor, h * 768 + ci * 256,
                                             [[1536, 128], [2, 128]])
                            nc.tensor.transpose(
                                out_ap,
                                dw8[ci][:, (b0 + h) * 128:(b0 + h + 1) * 128],
                                eye8[:])
                    qk = qkp.tile([128, 2, 384], FP8, tag="qk")
                    q3 = qk[:]
                    cp(plan["pair_q0"], q3[:, :, 0:192],
                       bass.AP(pt.tensor, 0, [[1536, 128], [768, 2], [2, 192]]))
                    cp(plan["pair_k2"], q3[:, :, 192:384],
                       bass.AP(pt.tensor, 384, [[1536, 128], [768, 2], [2, 192]]))
                    # software pipeline: PE never stalls on pack copies
                    depth = plan.get("pipe_depth", 2)
                    pending.append((qk, pb))
                    if len(pending) > depth:
                        pair_matmuls(*pending.pop(0))
                for args in pending:
                    pair_matmuls(*args)

            # ---- norms from Gram diagonals ----
            def diag_rs(gp_ap, mw, tmp_t, po=0):
                pe_ = slice(po, po + mw)
                scr = ap_.tile([128, 128], F32, tag="scr", bufs=2)
                nc.vector.tensor_tensor(scr[pe_, 0:mw], gp_ap,
                                        eyeb[pe_, po:po + mw], op=ALU.mult)
                d = sp.tile([128, 1], F32, tag="dg", bufs=4)
                nc.vector.tensor_reduce(d[pe_, :], scr[pe_, 0:mw],
                                        axis=AX.X, op=ALU.add)
                nc.scalar.activation(d[pe_, :], d[pe_, :], AF.Sqrt)
                nc.vector.tensor_scalar_max(d[pe_, :], d[pe_, :], EPS)
                rs = sp.tile([128, 1], F32, tag="rs", bufs=4)
                nc.vector.reciprocal(rs[pe_, :], d[pe_, :])
                if po:
                    # relocate to partitions 0..mw via tiny sbuf->sbuf DMA
                    rlo = sp.tile([128, 1], F32, tag="rs", bufs=4)
                    nc.sync.dma_start(rlo[0:mw, :], rs[pe_, :])
                    rs = rlo
                if tmp_t is not None:
                    nc.vector.tensor_tensor(rs[0:mw, :], rs[0:mw, :],
                                            tmp_t[0:mw, :], op=ALU.mult)
                return rs

            def d_rs(d_ap, mw, tmp_t, po=0):
                pe_ = slice(po, po + mw)
                dd = sp.tile([128, 1], F32, tag="dg", bufs=4)
                nc.scalar.activation(dd[pe_, :], d_ap, AF.Sqrt)
                nc.vector.tensor_scalar_max(dd[pe_, :], dd[pe_, :], EPS)
                rs = sp.tile([128, 1], F32, tag="rs", bufs=4)
                nc.vector.reciprocal(rs[pe_, :], dd[pe_, :])
                if po:
                    rlo = sp.tile([128, 1], F32, tag="rs", bufs=4)
                    nc.sync.dma_start(rlo[0:mw, :], rs[pe_, :])
                    rs = rlo
                if tmp_t is not None:
                    nc.vector.tensor_tensor(rs[0:mw, :], rs[0:mw, :],
                                            tmp_t[0:mw, :], op=ALU.mult)
                return rs

            rsq0 = diag_rs(gq0[:], 128, tmp0)
            rsq1 = d_rs(d1[0:64, :], 64, tmp1)
            rsk0 = diag_rs(gk0[:], 128, None)
            rsk1 = d_rs(d1[64:128, :], 64, None, po=64)

            # column scale: broadcast rsk over partitions via tiny PE matmuls
            nps_ctx = ExitStack()
            nps = nps_ctx.enter_context(
                tc.tile_pool(name="nps", bufs=1, space="PSUM"))
            rkb = sp.tile([128, 2], BF16, tag="rkb")
            nc.vector.tensor_copy(rkb[:, 0:1], rsk0[:])
            nc.vector.tensor_copy(rkb[0:64, 1:2], rsk1[0:64, :])
            ptk = nps.tile([1, 192], BF16, tag="npscr", name="ptk")
            nc.tensor.transpose(ptk[:, 0:128], rkb[:, 0:1], eyeb[:])
            nc.tensor.transpose(ptk[:, 128:192], rkb[0:64, 1:2],
                                eyeb[0:64, 0:64])
            rkrow = sp.tile([1, 192], BF16, tag="rkrow")
            nc.scalar.copy(rkrow[:], ptk[:])
            pbc = nps.tile([128, 192], F32, tag="npscr", name="pbc")
            nc.tensor.matmul(pbc[:], onesb[:], rkrow[:], start=True, stop=True)
            bcast = ap_.tile([128, 192], F32, tag="bcast")
            nc.scalar.copy(bcast[:], pbc[:])

            # ---- softmax + M^T ----
            def softmax_rows(apsum, rsq, msk, mw):
                a = ap_.tile([mw, C], F32, tag=f"a{mw}", name=f"a{mw}")
                nc.vector.tensor_tensor(a[:], apsum, bcast[0:mw, :],
                                        op=ALU.mult)
                nc.vector.scalar_tensor_tensor(
                    out=a[:], in0=a[:], scalar=rsq[0:mw, :], in1=msk[0:mw, :],
                    op0=ALU.mult, op1=ALU.add)
                mx = sp.tile([128, 1], F32, tag="mx", bufs=2)
                nc.vector.tensor_reduce(mx[0:mw, :], a[:], axis=AX.X, op=ALU.max)
                nmx = sp.tile([128, 1], F32, tag="nmx", bufs=2)
                nc.vector.tensor_scalar_mul(nmx[0:mw, :], mx[0:mw, :], -1.0)
                nc.scalar.activation(a[:], a[:], AF.Exp, bias=nmx[0:mw, :])
                sm = sp.tile([128, 1], F32, tag="sm", bufs=2)
                nc.vector.tensor_reduce(sm[0:mw, :], a[:], axis=AX.X, op=ALU.add)
                rsm = sp.tile([128, 1], F32, tag="rsm", bufs=2)
                nc.vector.reciprocal(rsm[0:mw, :], sm[0:mw, :])
                nc.vector.tensor_scalar_mul(a[:], a[:], rsm[0:mw, :])
                return a
            a0 = softmax_rows(apq0[:], rsq0, mask0, 128)
            a1 = softmax_rows(apq1, rsq1, mask1, 64)

            # M^T[d,o] = sum_c A[c,d] w_outT[c,o]
            for dlo, dw_, mt in ((0, 128, mt0), (128, 64, mt1)):
                pm = nps.tile([128, C], F32, tag="npscr", name="pm")
                nc.tensor.matmul(pm[0:dw_, :], a0[:, dlo:dlo + dw_],
                                 wo0[:], start=True, stop=False)
                nc.tensor.matmul(pm[0:dw_, :], a1[:, dlo:dlo + dw_],
                                 wo1[:], start=False, stop=True)
                nc.scalar.copy(mt[:], pm[0:dw_, :])
            nps_ctx.close()
            aps_ctx.close()

         # ================= phase 2: v in bf16 + y =================
         with tc.tile_pool(name="vprep", bufs=1) as vprep, \
             tc.tile_pool(name="v3p", bufs=1) as v3p, \
             tc.tile_pool(name="v4p", bufs=1) as v4p, \
             tc.tile_pool(name="xtp", bufs=4) as xtp, \
             tc.tile_pool(name="scp", bufs=2) as scp, \
             tc.tile_pool(name="gps2", bufs=2, space="PSUM") as gps, \
             tc.tile_pool(name="dps2", bufs=2, space="PSUM") as dps, \
             tc.tile_pool(name="yps", bufs=2, space="PSUM") as yps:

            vpre = vprep.tile([128, NPAD], BF16, tag="vpre")
            nc.gpsimd.memset(vpre[:, 0:PAD], 0.0)
            nc.gpsimd.memset(vpre[:, PAD + N:NPAD], 0.0)
            v3 = v3p.tile([128, N], BF16, tag="v3")
            v4 = v4p.tile([64, N], BF16, tag="v4")

            def fix_cases(ci):
                cases = []
                for dy in (-1, 0, 1):
                    cases.append(((dy + 1) * 3, max(0, 1 - dy),
                                  min(127, 128 - dy), 0, dy * W - 1))
                    cases.append(((dy + 1) * 3 + 2, max(0, -1 - dy),
                                  min(127, 126 - dy), W - 1, (dy + 1) * W))
                return cases

            def emit_fixes(ci, vt, vsrc, mw, yl, yh):
                """Edge fixes for image rows yl..yh of chunk ci."""
                for ti, y0, y1, oc, so in fix_cases(ci):
                    a, b = max(y0, yl), min(y1, yh)
                    if a > b:
                        continue
                    out_ap = vt[:, a * W + oc:b * W + oc + 1:W]
                    src_ap = bass.AP(vsrc.tensor, PAD + a * W + so,
                                     [[NPAD, mw], [W, b - a + 1]])
                    nc.vector.scalar_tensor_tensor(
                        out=out_ap, in0=src_ap,
                        scalar=wdn[0:mw, ci * 9 + ti:ci * 9 + ti + 1],
                        in1=out_ap, op0=ALU.mult, op1=ALU.add)

            def emit_y(t):
                cols = slice(t * NT, (t + 1) * NT)
                py0 = yps.tile([128, NT], F32, tag="y0")
                nc.tensor.matmul(py0[:], mt0[:, 0:128], v3[:, cols],
                                 start=True, stop=False)
                nc.tensor.matmul(py0[:], mt1[:, 0:128], v4[:, cols],
                                 start=False, stop=True, tile_position=(0, 0))
                y0 = yp.tile([128, NT], F32, tag="y0")
                cp(plan["y_copy"][t % 2], y0[:], py0[:])
                nc.sync.dma_start(y_d[0:128, cols], y0[:])
                py1 = yps.tile([64, NT], F32, tag="y1")
                nc.tensor.matmul(py1[:], mt0[:, 128:192], v3[:, cols],
                                 start=True, stop=False)
                nc.tensor.matmul(py1[:], mt1[:, 128:192], v4[:, cols],
                                 start=False, stop=True, tile_position=(0, 0))
                y1 = yp.tile([64, NT], F32, tag="y1")
                cp(plan["y_copy"][(t + 1) % 2], y1[:], py1[:])
                nc.sync.dma_start(y_d[128:192, cols], y1[:])
                return y0

            for k, (mw, vt, wdv, osl) in enumerate(
                    ((128, v3, wdv3, slice(0, 128)),
                     (64, v4, wdv4, slice(128, 192)))):
                ci = 3 + k
                for t in range(NTILES):
                    cols = slice(t * NT, (t + 1) * NT)
                    xt1 = xtp.tile([64, NT], BF16, tag="xt1")
                    nc.sync.dma_start(xt1[:], xt_d[:, cols])
                    pg = gps.tile([mw, NT], F32, tag="g")
                    nc.tensor.matmul(pg[:], wqv0[:, osl], xres0[:, cols],
                                     start=True, stop=False)
                    nc.tensor.matmul(pg[:], wqv1[:, osl], xt1[:],
                                     start=False, stop=True,
                                     tile_position=(0, 0))
                    cp(plan["v_pre"][t % 2],
                       vpre[0:mw, PAD + t * NT:PAD + (t + 1) * NT], pg[:])
                    if k == 1 and plan.get("dup4", True):
                        # partition-shifted dup of vpre4 on parts 64..127:
                        # dup[p+64, c] = vpre4[p, c+1]: one [128, NT] rhs
                        # feeds tap s (parts 0..63) and s+1 (parts 64..127)
                        c0 = max(0, t * NT - NT // 2 + PAD)
                        if t == 0:
                            nc.sync.dma_start(
                                bass.AP(vpre.tensor, 64 * NPAD,
                                        [[NPAD, 64], [1, PAD]]),
                                bass.AP(vpre.tensor, 1, [[NPAD, 64], [1, PAD]]))
                if k == 1 and plan.get("dup4", True):
                    for c0 in range(PAD, NPAD - 2, NT):
                        w_ = min(NT, NPAD - 2 - c0)
                        nc.sync.dma_start(
                            bass.AP(vpre.tensor, 64 * NPAD + c0,
                                    [[NPAD, 64], [1, w_]]),
                            bass.AP(vpre.tensor, c0 + 1, [[NPAD, 64], [1, w_]]))
                for t in range(NTILES):
                    base = t * NT
                    eng = plan["v_dw"](ci, t)
                    if eng == "pe" and k == 1 and plan.get("dup4", True):
                        pd = dps.tile([mw, NT], F32, tag="d")
                        for j, ti in enumerate((0, 3, 6)):   # paired K=128
                            nc.tensor.matmul(
                                pd[:], wdv4p[:, j * 64:(j + 1) * 64],
                                bass.AP(vpre.tensor, PAD + base + SHIFT[ti],
                                        [[NPAD, 128], [1, NT]]),
                                start=(j == 0), stop=False)
                        for j, ti in enumerate((2, 5, 8)):   # singles K=64
                            nc.tensor.matmul(
                                pd[:], wdv4p[0:64, (3 + j) * 64:(4 + j) * 64],
                                bass.AP(vpre.tensor, PAD + base + SHIFT[ti],
                                        [[NPAD, 64], [1, NT]]),
                                start=False, stop=(j == 2),
                                tile_position=(0, 0))
                        cp(plan["v_dw_copy"][t % 2],
                           vt[:, base:base + NT], pd[:])
                    elif eng == "pe":
                        pd = dps.tile([mw, NT], F32, tag="d")
                        for ti in range(9):
                            nc.tensor.matmul(
                                pd[:], wdv[0:mw, ti * 128:ti * 128 + mw],
                                bass.AP(vpre.tensor, PAD + base + SHIFT[ti],
                                        [[NPAD, mw], [1, NT]]),
                                start=(ti == 0), stop=(ti == 8))
                        cp(plan["v_dw_copy"][t % 2],
                           vt[:, base:base + NT], pd[:])
                    else:
                        # TSP (4x SIMD) + TT-add (2x) tree beats 1x STT chain
                        out = vt[:, base:base + NT]
                        for ti in range(9):
                            src = bass.AP(vpre.tensor,
                                          PAD + base + SHIFT[ti],
                                          [[NPAD, mw], [1, NT]])
                            wcol = wdvp[0:mw, k * 9 + ti:k * 9 + ti + 1]
                            if ti == 0:
                                nc.vector.tensor_scalar_mul(out, src, wcol)
                            else:
                                sc = scp.tile([128, NT], BF16, tag="sc")
                                nc.vector.tensor_scalar_mul(
                                    sc[0:mw, :], src, wcol)
                                nc.vector.tensor_tensor(
                                    out, out, sc[0:mw, :], op=ALU.add)
                    if k == 1:
                        # per-tile fixes let y stream right behind dw4
                        emit_fixes(4, v4, vpre, 64, 4 * t, 4 * t + 3)
                        if t >= 2:
                            emit_y(t - 2)
                if k == 0:
                    emit_fixes(3, v3, vpre, 128, 0, 127)
            emit_y(30)
            ylast = emit_y(31)

            if yprobe_d is not None:
                nc.sync.dma_start(yprobe_d[:], ylast[:, 0:4])

    nc.compile()
    return nc


def host_inputs(x, w_qkv, w_dw, w_out, temperature):
    """Host-side prep: per-core input maps."""
    b = x.shape[0]
    w9 = np.asarray(w_dw, np.float32).reshape(576, 9)
    wqT = np.ascontiguousarray(np.asarray(w_qkv, np.float32).T)  # [192, 576]

    # scales keep fp8 e4m3 (max finite 240) away from overflow:
    # pre = 4x true (sigma ~4), dw = 8x true (sigma ~8, max ~60 << 240)
    wq8 = np.zeros((128, 2, 384), np.float32)
    wq8[:, 0, :] = 4.0 * wqT[0:128, 0:384]
    wq8[0:64, 1, :] = 4.0 * wqT[128:192, 0:384]

    wdw8 = np.zeros((3, 128, 5, 2, 128), np.float32)
    rng = np.arange(128)
    for ci in range(3):
        for j, (ta, tb) in enumerate(QK_PAIRS):
            wdw8[ci, rng, j, 0, rng] = 2.0 * w9[ci * 128 + rng, ta]
            if tb is not None:
                wdw8[ci, rng, j, 1, rng] = 2.0 * w9[ci * 128 + rng, tb]

    wdwv = np.zeros((2, 128, 9 * 128), np.float32)
    for k, (s, wid) in enumerate(((384, 128), (512, 64))):
        for t in range(9):
            wdwv[k, :wid, t * 128:t * 128 + wid][
                np.arange(wid), np.arange(wid)] = w9[s:s + wid, t]

    wdn = np.zeros((128, 45), np.float32)
    for ci, (s, wid) in enumerate(
            ((0, 128), (128, 128), (256, 128), (384, 128), (512, 64))):
        sc = -2.0 if ci < 3 else -1.0
        wdn[:wid, ci * 9:(ci + 1) * 9] = sc * w9[s:s + wid, :]

    wdv4p = np.zeros((128, 6 * 64), np.float32)
    r64 = np.arange(64)
    for j, (ta, tb) in enumerate(((0, 1), (3, 4), (6, 7))):
        wdv4p[r64, j * 64 + r64] = w9[512 + r64, ta]
        wdv4p[64 + r64, j * 64 + r64] = w9[512 + r64, tb]
    for j, ts_ in enumerate((2, 5, 8)):
        wdv4p[r64, (3 + j) * 64 + r64] = w9[512 + r64, ts_]

    wdvp = np.zeros((128, 18), np.float32)
    wdvp[:, 0:9] = w9[384:512, :]
    wdvp[0:64, 9:18] = w9[512:576, :]

    temp_pc = np.repeat(np.asarray(temperature, np.float32).reshape(NHEADS),
                        HDIM).reshape(C, 1)
    mask = np.full((C, C), -1e9, np.float32)
    for h in range(NHEADS):
        mask[h * HDIM:(h + 1) * HDIM, h * HDIM:(h + 1) * HDIM] = 0.0

    f8 = ml_dtypes.float8_e4m3
    shared = {
        "wq8": wq8.reshape(128, 768).astype(f8).view(np.uint8),
        "wqv0": wqT[0:128, 384:576].astype(ml_dtypes.bfloat16),
        "wqv1": wqT[128:192, 384:576].astype(ml_dtypes.bfloat16),
        "wdw8": wdw8.reshape(3, 128, 1280).astype(f8).view(np.uint8),
        "wdwv": wdwv.astype(ml_dtypes.bfloat16),
        "wdv4p": wdv4p.astype(ml_dtypes.bfloat16),
        "wdn": wdn,
        "wdvp": wdvp,
        "w_outT": np.ascontiguousarray(np.asarray(w_out, np.float32).T),
        "temp": temp_pc,
        "mask": mask,
        "eye8": np.eye(128, dtype=np.float32).astype(f8).view(np.uint8),
        "eyeb": np.eye(128, dtype=ml_dtypes.bfloat16),
    }
    out = []
    for c in range(b):
        xc = np.ascontiguousarray(np.asarray(x[c], np.float32).reshape(C, N))
        out.append(dict(shared, x=xc,
                        xt16=xc[128:192].astype(ml_dtypes.bfloat16)))
    return out


_NC_CACHE = {}


def kernel(x, w_qkv, w_dw, w_out, temperature):
    x = np.asarray(x)
    if "nc" not in _NC_CACHE:
        _NC_CACHE["nc"] = build_nc()
    nc = _NC_CACHE["nc"]
    in_maps = host_inputs(x, w_qkv, w_dw, w_out, temperature)
    res = run_bass_kernel_spmd(nc, in_maps, list(range(8)))
    out = np.stack([res.results[c]["y"].reshape(C, H, W) for c in range(8)])
    return out.astype(np.float32)


# revision 32
# speedup vs baseline: 1.1458x; 1.1458x over previous
"""MDTA (Restormer transposed attention) Trainium2 Bass kernel.

Strategy: data-parallel over batch (8 cores x 1 image each). Per core:
  q,k path runs in fp8e4m3 with DoubleRow perf mode (0.5 cycles/row):
    1. qkv GEMM for q,k: one DR matmul per 512-col tile (K=192 packed as
       [128, 2, .] subtiles), weights x4 so fp8 stays in normal range.
    2. depthwise 3x3 for q,k: 9 taps paired into 5 DR diag matmuls per tile
       (pair deltas must be even), reading a zero-margin padded pre buffer;
       row-wrap contamination of dx=+-1 taps fixed by strided DVE STTs.
    3. streaming per 2-block pair: fp8 transposes (element-step-2 psum out)
       -> packed [128,2,192] q/k pair tiles -> DR matmuls accumulate raw
       attention logits AND q/k Gram matrices (norms = Gram diag, so all
       fp8/weight scalings cancel in q_hat/k_hat automatically).
    4. logits = G * rsq[c] * rsk[d] (row TSP + PE-broadcast column scale),
       mask + softmax, M^T = A^T w_outT (f32 matmuls).
  v path stays bf16 (its error reaches the output directly):
    5. v GEMM from resident bf16 x (K=64 tail streamed from DRAM),
       depthwise via diag matmuls on PE with some tiles offloaded to
       DVE/Pool STT chains, then y = M^T.T @ v_dw as one GEMM over v.
"""
import sys
sys.path.insert(0, "/opt/trn_rl_repo")
from contextlib import ExitStack

import numpy as np
import ml_dtypes

import concourse.bass as bass
import concourse.mybir as mybir
import concourse.tile as tile
from concourse import bacc
from concourse.bass_utils import run_bass_kernel_spmd

F32 = mybir.dt.float32
BF16 = mybir.dt.bfloat16
FP8 = mybir.dt.float8e4
AF = mybir.ActivationFunctionType
ALU = mybir.AluOpType
AX = mybir.AxisListType
DR = mybir.MatmulPerfMode.DoubleRow

C = 192
NHEADS = 8
HDIM = 24
H = W = 128
N = H * W            # 16384 spatial positions
NT = 512             # free-dim tile (4 image rows)
NTILES = N // NT     # 32
PAD = 256            # zero margin on each side of padded pre buffers
NPAD = N + 2 * PAD
EPS = 1e-12

# taps: index t = (dy+1)*3 + (dx+1), shift = dy*W + dx
TAPS = [(dy, dx) for dy in (-1, 0, 1) for dx in (-1, 0, 1)]
SHIFT = [dy * W + dx for dy, dx in TAPS]
# fp8 DR tap pairs (subtile deltas must be EVEN for walrus/HW):
#   (t_a, t_b) -> rhs offset SHIFT[t_a], delta SHIFT[t_b]-SHIFT[t_a]
QK_PAIRS = [(0, 2), (3, 5), (6, 8), (1, 4), (7, None)]

# engine plan (tunable): which engine handles each copy/offload site.
# NOTE: Pool/GPSIMD cannot access PSUM on HW -> psum->sbuf copies are
# ACT/DVE only; Pool gets SBUF-only work (STT dw chains, memsets).
PLAN = {
    "qk_pre": ["act", "dve"],            # striped by tile index
    "qk_dw": ["dve", "act"],
    "pair_q0": "act",                    # chunk0 pair pack copy [128,2,128]
    "pair_c1a": "act",                   # chunk1 q split copy [128,2,64]
    "pair_c1b": "dve",                   # chunk1 k split copy [128,2,64]
    "pair_k2": "dve",                    # chunk2 pair pack copy
    "v_pre": ["act", "act"],             # ACT (DVE runs dw trees)
    "v_dw_copy": ["act", "act"],
    "y_copy": ["act", "act"],
    # v dw tile offload: Pool lacks TensorScalarPtr on HW -> PE/DVE only
    "v_dw": lambda ci, t: ("dve" if t % 3 == 1 else "pe"),
    "pipe_depth": 2,                     # attn pair software-pipeline depth
    "dup4": False,                       # chunk4 tap-pairing via shifted dup
}


def build_nc(reps=1, abl=(), plan=PLAN):  # noqa: C901
    nc = bacc.Bacc("TRN2", target_bir_lowering=False, debug=False)
    x_d = (nc.dram_tensor("x_scratch", [C, N], F32) if "dummyx" in abl
           else nc.dram_tensor("x", [C, N], F32, kind="ExternalInput"))
    xt_d = (nc.dram_tensor("xt_scratch", [64, N], BF16) if "dummyx" in abl
            else nc.dram_tensor("xt16", [64, N], BF16, kind="ExternalInput"))
    wq8_d = nc.dram_tensor("wq8", [128, 768], FP8, kind="ExternalInput")
    wqv0_d = nc.dram_tensor("wqv0", [128, 192], BF16, kind="ExternalInput")
    wqv1_d = nc.dram_tensor("wqv1", [64, 192], BF16, kind="ExternalInput")
    wdw8_d = nc.dram_tensor("wdw8", [3, 128, 1280], FP8, kind="ExternalInput")
    wdwv_d = nc.dram_tensor("wdwv", [2, 128, 1152], BF16, kind="ExternalInput")
    wdv4p_d = nc.dram_tensor("wdv4p", [128, 384], BF16, kind="ExternalInput")
    wdn_d = nc.dram_tensor("wdn", [128, 45], F32, kind="ExternalInput")
    wdvp_d = nc.dram_tensor("wdvp", [128, 18], F32, kind="ExternalInput")
    wo_d = nc.dram_tensor("w_outT", [C, C], F32, kind="ExternalInput")
    tmp_d = nc.dram_tensor("temp", [C, 1], F32, kind="ExternalInput")
    mask_d = nc.dram_tensor("mask", [C, C], F32, kind="ExternalInput")
    eye8_d = nc.dram_tensor("eye8", [128, 128], FP8, kind="ExternalInput")
    eyeb_d = nc.dram_tensor("eyeb", [128, 128], BF16, kind="ExternalInput")
    if "dummyy" in abl:
        y_d = nc.dram_tensor("y_scratch", [C, N], F32)
        yprobe_d = nc.dram_tensor("y", [128, 4], F32, kind="ExternalOutput")
    else:
        y_d = nc.dram_tensor("y", [C, N], F32, kind="ExternalOutput")
        yprobe_d = None

    def cp(eng, out, in_):
        if eng == "act":
            nc.scalar.copy(out, in_)
        elif eng == "dve":
            nc.vector.tensor_copy(out, in_)
        else:
            nc.gpsimd.tensor_copy(out, in_)

    with tile.TileContext(nc) as tc, ExitStack() as ctx:
        wp = ctx.enter_context(tc.tile_pool(name="w", bufs=1))
        sp = ctx.enter_context(tc.tile_pool(name="small", bufs=1))
        yp = ctx.enter_context(tc.tile_pool(name="y", bufs=3))

        # ---- persistent weights ----
        wq8 = wp.tile([128, 2, 384], FP8, tag="wq8")
        nc.sync.dma_start(wq8[:].rearrange("p a b -> p (a b)"), wq8_d[:])
        wqv0 = wp.tile([128, 192], BF16, tag="wqv0")
        wqv1 = wp.tile([64, 192], BF16, tag="wqv1")
        nc.sync.dma_start(wqv0[:], wqv0_d[:])
        nc.sync.dma_start(wqv1[:], wqv1_d[:])
        wdw8 = []
        for ci in range(3):
            t_ = wp.tile([128, 5, 2, 128], FP8, tag=f"wdw8_{ci}", name=f"wdw8_{ci}")
            nc.sync.dma_start(t_[:].rearrange("p a b c -> p (a b c)"), wdw8_d[ci])
            wdw8.append(t_)
        wdv3 = wp.tile([128, 1152], BF16, tag="wdv3")
        nc.sync.dma_start(wdv3[:], wdwv_d[0])
        wdv4 = wp.tile([64, 1152], BF16, tag="wdv4")
        nc.sync.dma_start(wdv4[:], wdwv_d[1, 0:64, :])
        wdv4p = wp.tile([128, 384], BF16, tag="wdv4p")
        nc.sync.dma_start(wdv4p[:], wdv4p_d[:])
        wdn = wp.tile([128, 45], F32, tag="wdn")
        nc.sync.dma_start(wdn[:], wdn_d[:])
        wdvp = wp.tile([128, 18], F32, tag="wdvp")
        nc.sync.dma_start(wdvp[:], wdvp_d[:])
        wo0 = wp.tile([128, C], F32, tag="wo0")
        wo1 = wp.tile([64, C], F32, tag="wo1")
        nc.sync.dma_start(wo0[:], wo_d[0:128, :])
        nc.sync.dma_start(wo1[:], wo_d[128:192, :])
        tmp0 = wp.tile([128, 1], F32, tag="tmp0")
        tmp1 = wp.tile([64, 1], F32, tag="tmp1")
        nc.sync.dma_start(tmp0[:], tmp_d[0:128, :])
        nc.sync.dma_start(tmp1[:], tmp_d[128:192, :])
        mask0 = wp.tile([128, C], F32, tag="mask0")
        mask1 = wp.tile([64, C], F32, tag="mask1")
        nc.sync.dma_start(mask0[:], mask_d[0:128, :])
        nc.sync.dma_start(mask1[:], mask_d[128:192, :])
        eye8 = wp.tile([128, 128], FP8, tag="eye8")
        nc.sync.dma_start(eye8[:], eye8_d[:])
        eyeb = wp.tile([128, 128], BF16, tag="eyeb")
        nc.sync.dma_start(eyeb[:], eyeb_d[:])
        onesb = wp.tile([1, 128], BF16, tag="onesb")
        nc.vector.memset(onesb[:], 1.0)
        # M^T tiles (built in phase 1, used in phase 2)
        mt0 = wp.tile([128, C], BF16, tag="mt0")
        mt1 = wp.tile([64, C], BF16, tag="mt1")

        # resident x: bf16 (v path) + fp8 DR-packed (q,k path), loaded once
        xres0 = wp.tile([128, N], BF16, tag="xres0")
        xres8 = wp.tile([128, 2, N], FP8, tag="xres8")
        nc.gpsimd.memset(xres8[64:128, 1, :], 0.0)
        with tc.tile_pool(name="xload", bufs=3) as xp:
            for t in range(NTILES):
                cols = slice(t * NT, (t + 1) * NT)
                xt0 = xp.tile([128, NT], F32, tag="x0")
                xt1 = xp.tile([64, NT], F32, tag="x1")
                nc.sync.dma_start(xt0[:], x_d[0:128, cols])
                nc.sync.dma_start(xt1[:], x_d[128:192, cols])
                nc.vector.tensor_copy(xres0[:, cols], xt0[:])
                nc.scalar.copy(xres8[:, 0, cols], xt0[:])
                nc.gpsimd.tensor_copy(xres8[0:64, 1, cols], xt1[:])

        for _rep in range(reps):
         # ================= phase 1: q,k in fp8 =================
         with tc.tile_pool(name="pre8p", bufs=1) as pre8p, \
             tc.tile_pool(name="dw8p", bufs=1) as dw8p, \
             tc.tile_pool(name="qkp", bufs=4) as qkp, \
             tc.tile_pool(name="asb", bufs=1) as ap_:

            pre8 = pre8p.tile([128, NPAD], FP8, tag="pre8")
            nc.gpsimd.memset(pre8[:, 0:PAD], 0.0)
            nc.gpsimd.memset(pre8[:, PAD + N:NPAD], 0.0)
            dw8 = [dw8p.tile([128, N], FP8, tag=f"dw8_{ci}", name=f"dw8_{ci}")
                   for ci in range(3)]

            qk_psum = ExitStack()
            with qk_psum:
             gps = qk_psum.enter_context(
                 tc.tile_pool(name="gps1", bufs=2, space="PSUM"))
             dps = qk_psum.enter_context(
                 tc.tile_pool(name="dps1", bufs=2, space="PSUM"))
             for ci in range(3):
                # GEMM: one DR matmul per tile (K=192 packed)
                for t in range(NTILES):
                    cols = slice(t * NT, (t + 1) * NT)
                    pg = gps.tile([128, NT], F32, tag="g")
                    nc.tensor.matmul(pg[:], wq8[:, :, ci * 128:(ci + 1) * 128],
                                     xres8[:, :, cols], start=True, stop=True,
                                     perf_mode=DR)
                    cp(plan["qk_pre"][t % 2],
                       pre8[:, PAD + t * NT:PAD + (t + 1) * NT], pg[:])
                # depthwise: 5 DR matmuls per tile (paired taps)
                for t in range(NTILES):
                    base = t * NT
                    pd = dps.tile([128, NT], F32, tag="d")
                    for j, (ta, tb) in enumerate(QK_PAIRS):
                        off = PAD + base + SHIFT[ta]
                        delta = (SHIFT[tb] - SHIFT[ta]) if tb is not None else 2
                        rhs = bass.AP(pre8.tensor, off,
                                      [[NPAD, 128], [delta, 2], [1, NT]])
                        nc.tensor.matmul(pd[:], wdw8[ci][:, j], rhs,
                                         start=(j == 0), stop=(j == 4),
                                         perf_mode=DR)
                    cp(plan["qk_dw"][t % 2], dw8[ci][:, base:base + NT], pd[:])
                # edge fixes: subtract row-wrap contamination of dx=+-1 taps
                for dy in (-1, 0, 1):
                    ti_l = (dy + 1) * 3
                    y0, y1 = max(0, 1 - dy), min(127, 128 - dy)
                    out_ap = dw8[ci][:, y0 * W:y1 * W + 1:W]
                    src_ap = bass.AP(pre8.tensor, PAD + (y0 + dy) * W - 1,
                                     [[NPAD, 128], [W, y1 - y0 + 1]])
                    nc.vector.scalar_tensor_tensor(
                        out=out_ap, in0=src_ap,
                        scalar=wdn[:, ci * 9 + ti_l:ci * 9 + ti_l + 1],
                        in1=out_ap, op0=ALU.mult, op1=ALU.add)
                    ti_r = (dy + 1) * 3 + 2
                    y0, y1 = max(0, -1 - dy), min(127, 126 - dy)
                    out_ap = dw8[ci][:, y0 * W + W - 1:y1 * W + W:W]
                    src_ap = bass.AP(pre8.tensor, PAD + (y0 + dy + 1) * W,
                                     [[NPAD, 128], [W, y1 - y0 + 1]])
                    nc.vector.scalar_tensor_tensor(
                        out=out_ap, in0=src_ap,
                        scalar=wdn[:, ci * 9 + ti_r:ci * 9 + ti_r + 1],
                        in1=out_ap, op0=ALU.mult, op1=ALU.add)

            # ---- streaming transposes + raw attn + Gram accumulation ----
            # 6 concurrent psum accumulators; each needs its OWN bank
            # (interleaved start/stop chains sharing a bank corrupt on HW)
            aps_ctx = ExitStack()
            aps = aps_ctx.enter_context(
                tc.tile_pool(name="aps", bufs=1, space="PSUM"))
            apq0 = aps.tile([128, C], F32, tag="apq0")
            apq1t = aps.tile([64, C], F32, tag="apq1")
            apq1 = apq1t[:]
            gq0 = aps.tile([128, 128], F32, tag="gq0")
            gk0 = aps.tile([128, 128], F32, tag="gk0")
            gk1 = aps.tile([64, 64], F32, tag="gk1")

            def pair_matmuls(qk, pb):
                st, sp_ = (pb == 0), (pb == 63)
                q3 = qk[:]
                nc.tensor.matmul(apq0[:], q3[:, :, 0:128], q3[:, :, 192:384],
                                 start=st, stop=sp_, perf_mode=DR,
                                 skip_group_check=True)
                nc.tensor.matmul(apq1, q3[:, :, 128:192], q3[:, :, 192:384],
                                 start=st, stop=sp_, perf_mode=DR,
                                 skip_group_check=True)
                for out, lo, hi in ((gq0, 0, 128), (gk0, 192, 320),
                                    (gk1, 320, 384)):
                    nc.tensor.matmul(out[:], q3[:, :, lo:hi], q3[:, :, lo:hi],
                                     start=st, stop=sp_, perf_mode=DR,
                                     skip_group_check=True)

            junk = ap_.tile([64, 2048], BF16, tag="junk")
            stat = sp.tile([128, 8], F32, tag="stat")
            for s4 in range(8):
                nc.scalar.activation(junk[:],
                                     dw8[1][0:64, s4 * 2048:(s4 + 1) * 2048],
                                     AF.Square, accum_out=stat[0:64, s4:s4 + 1])
            d1 = sp.tile([128, 1], F32, tag="d1")
            nc.vector.tensor_reduce(d1[0:64, :], stat[0:64, :], axis=AX.X,
                                    op=ALU.add)
            pending = []
            with tc.tile_pool(name="tps", bufs=3, space="PSUM") as tps:
                for pb in range(64):
                    b0 = 2 * pb
                    # one psum tile holds all 3 chunk transposes (1536B<bank)
                    # h-major layout: (h, ci) at offset h*768+ci*256 makes
                    # the pack copy affine over all 384 channels
                    pt = tps.tile([128, 1536], FP8, tag="pt")
                    for ci in range(3):
                        for h in range(2):
                            out_ap = bass.AP(pt.tensor, h * 768 + ci * 256,
                                             [[1536, 128], [2, 128]])
                            nc.tensor.transpose(
                                out_ap,
                                dw8[ci][:, (b0 + h) * 128:(b0 + h + 1) * 128],
                                eye8[:])
                    qk = qkp.tile([128, 2, 384], FP8, tag="qk")
                    q3 = qk[:]
                    cp(plan["pair_q0"], q3[:, :, 0:192],
                       bass.AP(pt.tensor, 0, [[1536, 128], [768, 2], [2, 192]]))
                    cp(plan["pair_k2"], q3[:, :, 192:384],
                       bass.AP(pt.tensor, 384, [[1536, 128], [768, 2], [2, 192]]))
                    # software pipeline: PE never stalls on pack copies
                    depth = plan.get("pipe_depth", 2)
                    pending.append((qk, pb))
                    if len(pending) > depth:
                        pair_matmuls(*pending.pop(0))
                for args in pending:
                    pair_matmuls(*args)

            # ---- norms from Gram diagonals ----
            def diag_rs(gp_ap, mw, tmp_t, po=0):
                pe_ = slice(po, po + mw)
                scr = ap_.tile([128, 128], F32, tag="scr", bufs=2)
                nc.vector.tensor_tensor(scr[pe_, 0:mw], gp_ap,
                                        eyeb[pe_, po:po + mw], op=ALU.mult)
                d = sp.tile([128, 1], F32, tag="dg", bufs=4)
                nc.vector.tensor_reduce(d[pe_, :], scr[pe_, 0:mw],
                                        axis=AX.X, op=ALU.add)
                nc.scalar.activation(d[pe_, :], d[pe_, :], AF.Sqrt)
                nc.vector.tensor_scalar_max(d[pe_, :], d[pe_, :], EPS)
                rs = sp.tile([128, 1], F32, tag="rs", bufs=4)
                nc.vector.reciprocal(rs[pe_, :], d[pe_, :])
                if po:
                    # relocate to partitions 0..mw via tiny sbuf->sbuf DMA
                    rlo = sp.tile([128, 1], F32, tag="rs", bufs=4)
                    nc.sync.dma_start(rlo[0:mw, :], rs[pe_, :])
                    rs = rlo
                if tmp_t is not None:
                    nc.vector.tensor_tensor(rs[0:mw, :], rs[0:mw, :],
                                            tmp_t[0:mw, :], op=ALU.mult)
                return rs

            def d_rs(d_ap, mw, tmp_t, po=0):
                pe_ = slice(po, po + mw)
                dd = sp.tile([128, 1], F32, tag="dg", bufs=4)
                nc.scalar.activation(dd[pe_, :], d_ap, AF.Sqrt)
                nc.vector.tensor_scalar_max(dd[pe_, :], dd[pe_, :], EPS)
                rs = sp.tile([128, 1], F32, tag="rs", bufs=4)
                nc.vector.reciprocal(rs[pe_, :], dd[pe_, :])
                if po:
                    rlo = sp.tile([128, 1], F32, tag="rs", bufs=4)
                    nc.sync.dma_start(rlo[0:mw, :], rs[pe_, :])
                    rs = rlo
                if tmp_t is not None:
                    nc.vector.tensor_tensor(rs[0:mw, :], rs[0:mw, :],
                                            tmp_t[0:mw, :], op=ALU.mult)
                return rs

            rsq0 = diag_rs(gq0[:], 128, tmp0)
            rsq1 = d_rs(d1[0:64, :], 64, tmp1)
            rsk0 = diag_rs(gk0[:], 128, None)
            rsk1 = diag_rs(gk1[:], 64, None)

            # column scale: broadcast rsk over partitions via tiny PE matmuls
            nps_ctx = ExitStack()
            nps = nps_ctx.enter_context(
                tc.tile_pool(name="nps", bufs=1, space="PSUM"))
            rkb = sp.tile([128, 2], BF16, tag="rkb")
            nc.vector.tensor_copy(rkb[:, 0:1], rsk0[:])
            nc.vector.tensor_copy(rkb[0:64, 1:2], rsk1[0:64, :])
            ptk = nps.tile([1, 192], BF16, tag="npscr", name="ptk")
            nc.tensor.transpose(ptk[:, 0:128], rkb[:, 0:1], eyeb[:])
            nc.tensor.transpose(ptk[:, 128:192], rkb[0:64, 1:2],
                                eyeb[0:64, 0:64])
            rkrow = sp.tile([1, 192], BF16, tag="rkrow")
            nc.scalar.copy(rkrow[:], ptk[:])
            pbc = nps.tile([128, 192], F32, tag="npscr", name="pbc")
            nc.tensor.matmul(pbc[:], onesb[:], rkrow[:], start=True, stop=True)
            bcast = ap_.tile([128, 192], F32, tag="bcast")
            nc.scalar.copy(bcast[:], pbc[:])

            # ---- softmax + M^T ----
            def softmax_rows(apsum, rsq, msk, mw):
                a = ap_.tile([mw, C], F32, tag=f"a{mw}", name=f"a{mw}")
                nc.vector.tensor_tensor(a[:], apsum, bcast[0:mw, :],
                                        op=ALU.mult)
                nc.vector.scalar_tensor_tensor(
                    out=a[:], in0=a[:], scalar=rsq[0:mw, :], in1=msk[0:mw, :],
                    op0=ALU.mult, op1=ALU.add)
                mx = sp.tile([128, 1], F32, tag="mx", bufs=2)
                nc.vector.tensor_reduce(mx[0:mw, :], a[:], axis=AX.X, op=ALU.max)
                nmx = sp.tile([128, 1], F32, tag="nmx", bufs=2)
                nc.vector.tensor_scalar_mul(nmx[0:mw, :], mx[0:mw, :], -1.0)
                nc.scalar.activation(a[:], a[:], AF.Exp, bias=nmx[0:mw, :])
                sm = sp.tile([128, 1], F32, tag="sm", bufs=2)
                nc.vector.tensor_reduce(sm[0:mw, :], a[:], axis=AX.X, op=ALU.add)
                rsm = sp.tile([128, 1], F32, tag="rsm", bufs=2)
                nc.vector.reciprocal(rsm[0:mw, :], sm[0:mw, :])
                nc.vector.tensor_scalar_mul(a[:], a[:], rsm[0:mw, :])
                return a
            a0 = softmax_rows(apq0[:], rsq0, mask0, 128)
            a1 = softmax_rows(apq1, rsq1, mask1, 64)

            # M^T[d,o] = sum_c A[c,d] w_outT[c,o]
            for dlo, dw_, mt in ((0, 128, mt0), (128, 64, mt1)):
                pm = nps.tile([128, C], F32, tag="npscr", name="pm")
                nc.tensor.matmul(pm[0:dw_, :], a0[:, dlo:dlo + dw_],
                                 wo0[:], start=True, stop=False)
                nc.tensor.matmul(pm[0:dw_, :], a1[:, dlo:dlo + dw_],
                                 wo1[:], start=False, stop=True)
                nc.scalar.copy(mt[:], pm[0:dw_, :])
            nps_ctx.close()
            aps_ctx.close()

         # ================= phase 2: v in bf16 + y =================
         with tc.tile_pool(name="vprep", bufs=1) as vprep, \
             tc.tile_pool(name="v3p", bufs=1) as v3p, \
             tc.tile_pool(name="v4p", bufs=1) as v4p, \
             tc.tile_pool(name="xtp", bufs=4) as xtp, \
             tc.tile_pool(name="scp", bufs=2) as scp, \
             tc.tile_pool(name="gps2", bufs=2, space="PSUM") as gps, \
             tc.tile_pool(name="dps2", bufs=2, space="PSUM") as dps, \
             tc.tile_pool(name="yps", bufs=2, space="PSUM") as yps:

            vpre = vprep.tile([128, NPAD], BF16, tag="vpre")
            nc.gpsimd.memset(vpre[:, 0:PAD], 0.0)
            nc.gpsimd.memset(vpre[:, PAD + N:NPAD], 0.0)
            v3 = v3p.tile([128, N], BF16, tag="v3")
            v4 = v4p.tile([64, N], BF16, tag="v4")

            def fix_cases(ci):
                cases = []
                for dy in (-1, 0, 1):
                    cases.append(((dy + 1) * 3, max(0, 1 - dy),
                                  min(127, 128 - dy), 0, dy * W - 1))
                    cases.append(((dy + 1) * 3 + 2, max(0, -1 - dy),
                                  min(127, 126 - dy), W - 1, (dy + 1) * W))
                return cases

            def emit_fixes(ci, vt, vsrc, mw, yl, yh):
                """Edge fixes for image rows yl..yh of chunk ci."""
                for ti, y0, y1, oc, so in fix_cases(ci):
                    a, b = max(y0, yl), min(y1, yh)
                    if a > b:
                        continue
                    out_ap = vt[:, a * W + oc:b * W + oc + 1:W]
                    src_ap = bass.AP(vsrc.tensor, PAD + a * W + so,
                                     [[NPAD, mw], [W, b - a + 1]])
                    nc.vector.scalar_tensor_tensor(
                        out=out_ap, in0=src_ap,
                        scalar=wdn[0:mw, ci * 9 + ti:ci * 9 + ti + 1],
                        in1=out_ap, op0=ALU.mult, op1=ALU.add)

            def emit_y(t):
                cols = slice(t * NT, (t + 1) * NT)
                py0 = yps.tile([128, NT], F32, tag="y0")
                nc.tensor.matmul(py0[:], mt0[:, 0:128], v3[:, cols],
                                 start=True, stop=False)
                nc.tensor.matmul(py0[:], mt1[:, 0:128], v4[:, cols],
                                 start=False, stop=True, tile_position=(0, 0))
                y0 = yp.tile([128, NT], F32, tag="y0")
                cp(plan["y_copy"][t % 2], y0[:], py0[:])
                nc.sync.dma_start(y_d[0:128, cols], y0[:])
                py1 = yps.tile([64, NT], F32, tag="y1")
                nc.tensor.matmul(py1[:], mt0[:, 128:192], v3[:, cols],
                                 start=True, stop=False)
                nc.tensor.matmul(py1[:], mt1[:, 128:192], v4[:, cols],
                                 start=False, stop=True, tile_position=(0, 0))
                y1 = yp.tile([64, NT], F32, tag="y1")
                cp(plan["y_copy"][(t + 1) % 2], y1[:], py1[:])
                nc.sync.dma_start(y_d[128:192, cols], y1[:])
                return y0

            for k, (mw, vt, wdv, osl) in enumerate(
                    ((128, v3, wdv3, slice(0, 128)),
                     (64, v4, wdv4, slice(128, 192)))):
                ci = 3 + k
                for t in range(NTILES):
                    cols = slice(t * NT, (t + 1) * NT)
                    xt1 = xtp.tile([64, NT], BF16, tag="xt1")
                    nc.sync.dma_start(xt1[:], xt_d[:, cols])
                    pg = gps.tile([mw, NT], F32, tag="g")
                    nc.tensor.matmul(pg[:], wqv0[:, osl], xres0[:, cols],
                                     start=True, stop=False)
                    nc.tensor.matmul(pg[:], wqv1[:, osl], xt1[:],
                                     start=False, stop=True,
                                     tile_position=(0, 0))
                    cp(plan["v_pre"][t % 2],
                       vpre[0:mw, PAD + t * NT:PAD + (t + 1) * NT], pg[:])
                    if k == 1 and plan.get("dup4", True):
                        # partition-shifted dup of vpre4 on parts 64..127:
                        # dup[p+64, c] = vpre4[p, c+1]: one [128, NT] rhs
                        # feeds tap s (parts 0..63) and s+1 (parts 64..127)
                        c0 = max(0, t * NT - NT // 2 + PAD)
                        if t == 0:
                            nc.sync.dma_start(
                                bass.AP(vpre.tensor, 64 * NPAD,
                                        [[NPAD, 64], [1, PAD]]),
                                bass.AP(vpre.tensor, 1, [[NPAD, 64], [1, PAD]]))
                if k == 1 and plan.get("dup4", True):
                    for c0 in range(PAD, NPAD - 2, NT):
                        w_ = min(NT, NPAD - 2 - c0)
                        nc.sync.dma_start(
                            bass.AP(vpre.tensor, 64 * NPAD + c0,
                                    [[NPAD, 64], [1, w_]]),
                            bass.AP(vpre.tensor, c0 + 1, [[NPAD, 64], [1, w_]]))
                for t in range(NTILES):
                    base = t * NT
                    eng = plan["v_dw"](ci, t)
                    if eng == "pe" and k == 1 and plan.get("dup4", True):
                        pd = dps.tile([mw, NT], F32, tag="d")
                        for j, ti in enumerate((0, 3, 6)):   # paired K=128
                            nc.tensor.matmul(
                                pd[:], wdv4p[:, j * 64:(j + 1) * 64],
                                bass.AP(vpre.tensor, PAD + base + SHIFT[ti],
                                        [[NPAD, 128], [1, NT]]),
                                start=(j == 0), stop=False)
                        for j, ti in enumerate((2, 5, 8)):   # singles K=64
                            nc.tensor.matmul(
                                pd[:], wdv4p[0:64, (3 + j) * 64:(4 + j) * 64],
                                bass.AP(vpre.tensor, PAD + base + SHIFT[ti],
                                        [[NPAD, 64], [1, NT]]),
                                start=False, stop=(j == 2),
                                tile_position=(0, 0))
                        cp(plan["v_dw_copy"][t % 2],
                           vt[:, base:base + NT], pd[:])
                    elif eng == "pe":
                        pd = dps.tile([mw, NT], F32, tag="d")
                        for ti in range(9):
                            nc.tensor.matmul(
                                pd[:], wdv[0:mw, ti * 128:ti * 128 + mw],
                                bass.AP(vpre.tensor, PAD + base + SHIFT[ti],
                                        [[NPAD, mw], [1, NT]]),
                                start=(ti == 0), stop=(ti == 8))
                        cp(plan["v_dw_copy"][t % 2],
                           vt[:, base:base + NT], pd[:])
                    else:
                        # TSP (4x SIMD) + TT-add (2x) tree beats 1x STT chain
                        out = vt[:, base:base + NT]
                        for ti in range(9):
                            src = bass.AP(vpre.tensor,
                                          PAD + base + SHIFT[ti],
                                          [[NPAD, mw], [1, NT]])
                            wcol = wdvp[0:mw, k * 9 + ti:k * 9 + ti + 1]
                            if ti == 0:
                                nc.vector.tensor_scalar_mul(out, src, wcol)
                            else:
                                sc = scp.tile([128, NT], BF16, tag="sc")
                                nc.vector.tensor_scalar_mul(
                                    sc[0:mw, :], src, wcol)
                                nc.vector.tensor_tensor(
                                    out, out, sc[0:mw, :], op=ALU.add)
                    if k == 1:
                        # per-tile fixes let y stream right behind dw4
                        emit_fixes(4, v4, vpre, 64, 4 * t, 4 * t + 3)
                        if t >= 2:
                            emit_y(t - 2)
                if k == 0:
                    emit_fixes(3, v3, vpre, 128, 0, 127)
            emit_y(30)
            ylast = emit_y(31)

            if yprobe_d is not None:
                nc.sync.dma_start(yprobe_d[:], ylast[:, 0:4])

    nc.compile()
    return nc


def host_inputs(x, w_qkv, w_dw, w_out, temperature):
    """Host-side prep: per-core input maps."""
    b = x.shape[0]
    w9 = np.asarray(w_dw, np.float32).reshape(576, 9)
    wqT = np.ascontiguousarray(np.asarray(w_qkv, np.float32).T)  # [192, 576]

    # scales keep fp8 e4m3 (max finite 240) away from overflow:
    # pre = 4x true (sigma ~4), dw = 8x true (sigma ~8, max ~60 << 240)
    wq8 = np.zeros((128, 2, 384), np.float32)
    wq8[:, 0, :] = 4.0 * wqT[0:128, 0:384]
    wq8[0:64, 1, :] = 4.0 * wqT[128:192, 0:384]

    wdw8 = np.zeros((3, 128, 5, 2, 128), np.float32)
    rng = np.arange(128)
    for ci in range(3):
        for j, (ta, tb) in enumerate(QK_PAIRS):
            wdw8[ci, rng, j, 0, rng] = 2.0 * w9[ci * 128 + rng, ta]
            if tb is not None:
                wdw8[ci, rng, j, 1, rng] = 2.0 * w9[ci * 128 + rng, tb]

    wdwv = np.zeros((2, 128, 9 * 128), np.float32)
    for k, (s, wid) in enumerate(((384, 128), (512, 64))):
        for t in range(9):
            wdwv[k, :wid, t * 128:t * 128 + wid][
                np.arange(wid), np.arange(wid)] = w9[s:s + wid, t]

    wdn = np.zeros((128, 45), np.float32)
    for ci, (s, wid) in enumerate(
            ((0, 128), (128, 128), (256, 128), (384, 128), (512, 64))):
        sc = -2.0 if ci < 3 else -1.0
        wdn[:wid, ci * 9:(ci + 1) * 9] = sc * w9[s:s + wid, :]

    wdv4p = np.zeros((128, 6 * 64), np.float32)
    r64 = np.arange(64)
    for j, (ta, tb) in enumerate(((0, 1), (3, 4), (6, 7))):
        wdv4p[r64, j * 64 + r64] = w9[512 + r64, ta]
        wdv4p[64 + r64, j * 64 + r64] = w9[512 + r64, tb]
    for j, ts_ in enumerate((2, 5, 8)):
        wdv4p[r64, (3 + j) * 64 + r64] = w9[512 + r64, ts_]

    wdvp = np.zeros((128, 18), np.float32)
    wdvp[:, 0:9] = w9[384:512, :]
    wdvp[0:64, 9:18] = w9[512:576, :]

    temp_pc = np.repeat(np.asarray(temperature, np.float32).reshape(NHEADS),
                        HDIM).reshape(C, 1)
    mask = np.full((C, C), -1e9, np.float32)
    for h in range(NHEADS):
        mask[h * HDIM:(h + 1) * HDIM, h * HDIM:(h + 1) * HDIM] = 0.0

    f8 = ml_dtypes.float8_e4m3
    shared = {
        "wq8": wq8.reshape(128, 768).astype(f8).view(np.uint8),
        "wqv0": wqT[0:128, 384:576].astype(ml_dtypes.bfloat16),
        "wqv1": wqT[128:192, 384:576].astype(ml_dtypes.bfloat16),
        "wdw8": wdw8.reshape(3, 128, 1280).astype(f8).view(np.uint8),
        "wdwv": wdwv.astype(ml_dtypes.bfloat16),
        "wdv4p": wdv4p.astype(ml_dtypes.bfloat16),
        "wdn": wdn,
        "wdvp": wdvp,
        "w_outT": np.ascontiguousarray(np.asarray(w_out, np.float32).T),
        "temp": temp_pc,
        "mask": mask,
        "eye8": np.eye(128, dtype=np.float32).astype(f8).view(np.uint8),
        "eyeb": np.eye(128, dtype=ml_dtypes.bfloat16),
    }
    out = []
    for c in range(b):
        xc = np.ascontiguousarray(np.asarray(x[c], np.float32).reshape(C, N))
        out.append(dict(shared, x=xc,
                        xt16=xc[128:192].astype(ml_dtypes.bfloat16)))
    return out


_NC_CACHE = {}


def kernel(x, w_qkv, w_dw, w_out, temperature):
    x = np.asarray(x)
    if "nc" not in _NC_CACHE:
        _NC_CACHE["nc"] = build_nc()
    nc = _NC_CACHE["nc"]
    in_maps = host_inputs(x, w_qkv, w_dw, w_out, temperature)
    res = run_bass_kernel_spmd(nc, in_maps, list(range(8)))
    out = np.stack([res.results[c]["y"].reshape(C, H, W) for c in range(8)])
    return out.astype(np.float32)
